# revision 13
# baseline (speedup 1.0000x reference)
"""Trainium2 Bass/Tile kernel for nn_MultiHeadAttention (B=4, S=2048, D=1024,
H=16, Dh=64, fp32), SPMD across 8 NeuronCores.

Sharding: core c -> batch c//2, head-half c%2 (8 heads per core).
Host pre-transposes each batch slice to [D, S], packs the weight slices to
[128, KT8*JW] (one 8KB-contiguous DMA row per partition instead of 1KB
bursts), and casts to bf16.  On device the QK projections produce Q^T/K^T
[feat, tok], the V projection produces V [tok, feat] with an appended
ones-column, scores come out as scores^T [k, q] (head pairs run
concurrently on the PE via 64-row tile positions), exp runs on the scalar
engine (scale 1/sqrt(Dh) folded in; scores bounded ~+-3 so no
max-subtraction), and the PV matmul uses V as stationary, yielding out^T
plus the softmax denominator from the ones column.  Host divides by the
denominator, adds the V bias, transposes, reassembles.

Schedule: the kernel is ACT(exp)-bound (~33.6M exps/core at 1 elem/lane/cyc
@1.2GHz plus ~430 cycles per instruction), so everything is organized
around keeping the scalar engine streaming:
  - scores ring: 2 PSUM tiles x 3 banks ([128, 3, 512] f32); each ACTIVATE
    covers 3 slabs (1536 elem/lane) to amortize the per-instruction cost.
  - score matmuls are emitted kt-pair at a time with ACTIVATEs deferred to
    kt boundaries, so the two 64-row head matmuls of each kt stay adjacent
    in the PE queue and execute concurrently (row tile positions 0/64).
  - PV chains are split into 4-matmul quarters emitted between score
    groups; projection work (incl. the next head-pair's, with its input
    re-DMA) is chopped into ~1us filler items consumed in leftover PE
    slots, with drain barriers before any emission that reads their output
    (emission order defines the dependency graph).
  - V-projection is computed in half-width chunks (head pairs 0-1 first,
    2-3 during pair 1) so the first PV chains aren't gated on the full V.
  - startup splits the inputs across both HW-DGE queues (k+v on sync, q on
    scalar, [128,1024] half tiles) and the exp table is preloaded.
PSUM: scores 6 banks, PV accumulator 1 bank, projection accumulator 1 bank
(shared with the final qt's second PV chain).
SBUF: Q^T/K^T live in a 2-slot ring (current pair + next being projected).
"""

import collections

import numpy as np
import ml_dtypes

import concourse.bacc as bacc
import concourse.tile as tile
from concourse import mybir
from concourse.bass_utils import run_bass_kernel_spmd

F32 = mybir.dt.float32
BF16 = mybir.dt.bfloat16
_BF = ml_dtypes.bfloat16

B, S, D, H, DH = 4, 2048, 1024, 16, 64
HH = 8          # heads per core
NP = HH // 2    # head pairs per core
JW = HH * DH    # 512 projected features per core
N_CORES = 8

KT8 = D // 128   # 8 contraction tiles for projections
NQT = S // 512   # 4 q-chunks of 512
NKT = S // 128   # 16 k-blocks of 128
NTT = S // 128   # 16 v token blocks
TC = 512         # projection token chunk
NTC = S // TC
HTOK = 1024      # k/q half-tile token width
GS = 3           # slabs per score PSUM tile / per ACTIVATE
NSLAB = 2 * NKT  # 32 score slabs per (pair, qt)
NG = (NSLAB + GS - 1) // GS   # 11 score tiles per (pair, qt)


def _build_nc(exp_bufs=18, in_bufs=28):
    nc = bacc.Bacc("TRN2", target_bir_lowering=False, debug=False,
                   num_devices=N_CORES)

    qT = nc.declare_dram_parameter("qT", [D, S], BF16, isOutput=False)
    kT = nc.declare_dram_parameter("kT", [D, S], BF16, isOutput=False)
    vT = nc.declare_dram_parameter("vT", [D, S], BF16, isOutput=False)
    # host-packed: [128, KT8*JW], row p = concat over kt of W[kt*128+p, :]
    wq = nc.declare_dram_parameter("wq", [128, KT8 * JW], BF16, isOutput=False)
    wk = nc.declare_dram_parameter("wk", [128, KT8 * JW], BF16, isOutput=False)
    wv = nc.declare_dram_parameter("wv", [128, KT8 * JW], BF16, isOutput=False)
    bq = nc.declare_dram_parameter("bq", [JW], F32, isOutput=False)
    bk = nc.declare_dram_parameter("bk", [JW], F32, isOutput=False)
    numT = nc.declare_dram_parameter("numT", [HH, 65, S], F32, isOutput=True)
    w_dram = {"wq": wq, "wk": wk, "wv": wv}
    in_dram = {"q": qT, "k": kT}

    with tile.TileContext(nc) as tc:
        with (
            tc.tile_pool(name="consts", bufs=1) as consts,
            tc.tile_pool(name="persist", bufs=1) as persist,
            tc.tile_pool(name="ins", bufs=in_bufs) as ins,
            tc.tile_pool(name="vins", bufs=8) as vins,
            tc.tile_pool(name="exps", bufs=exp_bufs) as exps,
            tc.tile_pool(name="ostage", bufs=4) as ostage,
            tc.tile_pool(name="scps", bufs=2, space="PSUM") as scps,
            tc.tile_pool(name="pvps", bufs=1, space="PSUM") as pvps,
            tc.tile_pool(name="prps", bufs=1, space="PSUM") as prps,
        ):
            w_sb = {}

            def load_w(name, eng=None):
                t = consts.tile([128, KT8, JW], BF16, tag=name)
                (eng or nc.sync).dma_start(
                    out=t[:].rearrange("p a b -> p (a b)"),
                    in_=w_dram[name].ap())
                w_sb[name] = t

            def load_bias(name, src):
                t = consts.tile([128, NP], F32, tag=name)
                nc.sync.dma_start(
                    out=t[:], in_=src.ap().rearrange("(pr j) -> j pr", j=128))
                return t

            # 2-slot ring: slot pair%2 holds that pair's projections
            QT_sb = persist.tile([128, 2, S], BF16, tag="QT")
            KT_sb = persist.tile([128, 2, S], BF16, tag="KT")
            V_aug = persist.tile([128, NTT, HH, 65], BF16, tag="Vaug")

            def load_half(name, kt, half, eng=None):
                t = ins.tile([128, HTOK], BF16, tag="in")
                (eng or nc.sync).dma_start(
                    out=t[:],
                    in_=in_dram[name].ap()[kt * 128:(kt + 1) * 128,
                                           half * HTOK:(half + 1) * HTOK])
                return t

            def load_vfull(kt):
                t = vins.tile([128, S], BF16, tag="vin")
                nc.sync.dma_start(
                    out=t[:], in_=vT.ap()[kt * 128:(kt + 1) * 128, :])
                return t

            # ---- ACT exp-table warm-up ----
            warm = persist.tile([128, 2], BF16, tag="warm")
            nc.vector.memset(warm[:, 0:1], 0.0)
            nc.scalar.activation(warm[:, 1:2], warm[:, 0:1],
                                 mybir.ActivationFunctionType.Exp, scale=0.125)

            # ---------------- projections ----------------
            proj_ps = {}

            def proj_qk_chunk(pair, name, s, halves, part):
                """part 0: MMs kt0-3; part 1: MMs kt4-7 + bias add."""
                wname, dst = {"k": ("wk", KT_sb), "q": ("wq", QT_sb)}[name]
                bias = bias_k if name == "k" else bias_q
                tc0 = s * TC
                hf, off = divmod(tc0, HTOK)
                if part == 0:
                    t = prps.tile([128, TC], F32, tag="pr",
                                  name=f"ps_{pair}_{name}_{s}")
                    proj_ps[(pair, name, s)] = t
                else:
                    t = proj_ps.pop((pair, name, s))
                for kt in range(part * 4, part * 4 + 4):
                    nc.tensor.matmul(
                        t[:], w_sb[wname][:, kt, pair * 128:(pair + 1) * 128],
                        halves[(kt, hf)][:, off:off + TC],
                        start=(kt == 0), stop=(kt == KT8 - 1))
                if part == 1:
                    nc.vector.tensor_scalar_add(
                        dst[:, pair % 2, tc0:tc0 + TC], t[:],
                        bias[:, pair:pair + 1])

            def proj_v_chunk(tt, ph, v_tiles):
                """V chunk for token block tt, head pairs 2ph..2ph+1."""
                t = prps.tile([128, 2 * 128], F32, tag="pr",
                              name=f"psv_{tt}_{ph}")
                for kt in range(KT8):
                    nc.tensor.matmul(
                        t[:], v_tiles[kt][:, tt * 128:(tt + 1) * 128],
                        w_sb["wv"][:, kt, ph * 256:(ph + 1) * 256],
                        start=(kt == 0), stop=(kt == KT8 - 1))
                nc.vector.tensor_copy(
                    V_aug[:, tt, ph * 4:(ph + 1) * 4, 0:64],
                    t[:].rearrange("p (h d) -> p h d", d=64))

            # ---------------- filler queue with drain barriers ----------
            filler = collections.deque()
            done_keys = set()

            def fill(budget_ns):
                while filler and budget_ns >= filler[0][0]:
                    cost, key, fn = filler.popleft()
                    fn()
                    if key:
                        done_keys.add(key)
                    budget_ns -= cost

            def drain(key):
                if key in done_keys:
                    return
                while filler:
                    cost, k, fn = filler.popleft()
                    fn()
                    if k:
                        done_keys.add(k)
                    if k == key:
                        return
                raise RuntimeError(f"drain: key {key} not found")

            def drain_all():
                while filler:
                    cost, k, fn = filler.popleft()
                    fn()
                    if k:
                        done_keys.add(k)

            # ---------------- attention ----------------
            et_map = {}     # (pair, qt, kt, h2) -> (et_tile, slot)
            pv_state = {}

            def pv_quarter(pair, qt, h2, quarter, pool=None):
                h = pair * 2 + h2
                q0 = qt * 512
                for kt in range(quarter * 4, quarter * 4 + 4):
                    drain(("vh", pair // 2, kt))
                if quarter == 0:
                    pv = (pool or pvps).tile(
                        [65, 512], F32, tag="pr" if pool else "pv",
                        name=f"pv_{pair}_{qt}_{h2}")
                    pv_state[(pair, qt, h2)] = pv
                else:
                    pv = pv_state[(pair, qt, h2)]
                for kt in range(quarter * 4, quarter * 4 + 4):
                    et, slot = et_map[(pair, qt, kt, h2)]
                    nc.tensor.matmul(
                        pv[:], V_aug[:, kt, h, :], et[:, slot, :],
                        start=(kt == 0), stop=(kt == NKT - 1))
                if quarter == 3:
                    del pv_state[(pair, qt, h2)]
                    ot = ostage.tile([65, 512], F32, tag="ot")
                    nc.vector.tensor_copy(ot[:], pv[:])
                    nc.sync.dma_start(
                        out=numT.ap()[h, :, q0:q0 + 512], in_=ot[:])
                    for kt in range(NKT):
                        del et_map[(pair, qt, kt, h2)]

            def emit_act(pair, qt, g, sc, n):
                et = exps.tile([128, GS, 512], BF16, tag="exp")
                nc.scalar.activation(
                    et[:, 0:n, :].rearrange("p a b -> p (a b)"),
                    sc[:].rearrange("p a b -> p (a b)"),
                    mybir.ActivationFunctionType.Exp, scale=0.125)
                for jj in range(n):
                    kk, hh2 = divmod(g * GS + jj, 2)
                    et_map[(pair, qt, kk, hh2)] = (et, jj)

            def attn_qt(pair, qt, prev, own_pv_tail=False):
                """Score matmuls kt-pair at a time (keeps the h0/h1 64-row
                matmuls adjacent -> concurrent); ACTIVATEs fire at kt
                boundaries; PV quarters of `prev` + filler fill the PE."""
                q0 = qt * 512
                drain(("q", pair, qt))
                sc_tiles = {}
                acted = 0
                slot = 0

                def do_slot():
                    nonlocal slot
                    if prev is not None and slot < 8:
                        pv_quarter(prev[0], prev[1], slot // 4, slot % 4)
                        fill(950)
                    elif own_pv_tail and slot >= 8:
                        qi = slot - 8
                        pv_quarter(pair, qt, 0, qi)
                        pv_quarter(pair, qt, 1, qi, pool=prps)
                    else:
                        fill(1250)
                    slot += 1

                for kt in range(NKT):
                    # ACTs for tiles completed by earlier kts, then PV +
                    # filler, then this kt's score pair -- so when the
                    # score matmuls block on the scps ring (ACT-bound
                    # stretches), the PV/filler work sits AHEAD of them in
                    # the in-order PE queue instead of behind.
                    completed = (2 * kt) // GS
                    while acted < completed:
                        sc, n = sc_tiles.pop(acted)
                        emit_act(pair, qt, acted, sc, n)
                        acted += 1
                        do_slot()
                    if kt % 4 == 0:
                        drain(("k", pair, kt // 4))
                    for h2 in range(2):
                        s = 2 * kt + h2
                        g, j = divmod(s, GS)
                        if j == 0:
                            n = min(GS, NSLAB - g * GS)
                            sc_tiles[g] = (scps.tile(
                                [128, n, 512], F32, tag="sc",
                                name=f"sc_{pair}_{qt}_{g}"), n)
                        nc.tensor.matmul(
                            sc_tiles[g][0][:, j, :],
                            KT_sb[h2 * 64:(h2 + 1) * 64, pair % 2,
                                  kt * 128:(kt + 1) * 128],
                            QT_sb[h2 * 64:(h2 + 1) * 64, pair % 2,
                                  q0:q0 + 512],
                            start=True, stop=True)
                while acted < NG:
                    sc, n = sc_tiles.pop(acted)
                    emit_act(pair, qt, acted, sc, n)
                    acted += 1
                    do_slot()

            # ---------------- emission ----------------
            # Keep both HW-DGE queues' startup bursts small: the DMA ring
            # credits are shared, so a long sync burst starves the scalar
            # queue's q stream (observed: q delayed ~15us behind k+v).
            # sync: wk + k half-0 only; scalar: wq + q half-0.  Everything
            # else (k half-1, wv, v, q half-1) issues as early filler items
            # on sync, in deadline order.
            load_w("wk")
            k_halves = {}
            for kt in range(KT8):
                k_halves[(kt, 0)] = load_half("k", kt, 0)
            bias_q = load_bias("bq", bq)
            bias_k = load_bias("bk", bk)
            load_w("wq", eng=nc.scalar)
            q_halves = {}
            for kt in range(KT8):
                q_halves[(kt, 0)] = load_half("q", kt, 0, eng=nc.scalar)
            v_tiles = {}

            nc.vector.memset(V_aug[:, :, :, 64:65], 1.0)

            # direct: K chunk s0 + Q chunk s0 -> first scores
            for part in range(2):
                proj_qk_chunk(0, "k", 0, k_halves, part)
            for part in range(2):
                proj_qk_chunk(0, "q", 0, q_halves, part)
            done_keys.add(("k", 0, 0))
            done_keys.add(("q", 0, 0))

            def push_projqk(pair, name, s, tiles):
                for part in range(2):
                    filler.append(
                        (950, (name, pair, s) if part else None,
                         lambda s=s, part=part, tiles=tiles, pair=pair,
                         name=name:
                         proj_qk_chunk(pair, name, s, tiles, part)))

            def push_projv(ph):
                for tt in range(NTT):
                    filler.append(
                        (700, ("vh", ph, tt),
                         lambda tt=tt, ph=ph: proj_v_chunk(tt, ph, v_tiles)))

            # pair-0 filler, deadline order: K s1 (k half-0), k half-1 DMAs,
            # K s2/s3, wv + v DMAs, Q s1, V half 0 chunks, q half-1 DMAs,
            # Q s2, Q s3
            push_projqk(0, "k", 1, k_halves)
            for kt in range(KT8):
                filler.append(
                    (80, None, lambda kt=kt:
                     k_halves.__setitem__((kt, 1), load_half("k", kt, 1))))
            push_projqk(0, "k", 2, k_halves)
            push_projqk(0, "k", 3, k_halves)
            filler.append((80, None, lambda: load_w("wv")))
            for kt in range(KT8):
                filler.append(
                    (80, None, lambda kt=kt:
                     v_tiles.__setitem__(kt, load_vfull(kt))))
            push_projqk(0, "q", 1, q_halves)
            push_projv(0)
            for kt in range(KT8):
                filler.append(
                    (80, None, lambda kt=kt:
                     q_halves.__setitem__((kt, 1), load_half("q", kt, 1))))
            push_projqk(0, "q", 2, q_halves)
            push_projqk(0, "q", 3, q_halves)

            def queue_pair_prep(p):
                """Input re-DMA + K/Q projections for pair p (filler)."""
                kh, qh = {}, {}
                for hf in range(2):
                    for kt in range(KT8):
                        filler.append(
                            (80, None, lambda kt=kt, hf=hf:
                             kh.__setitem__((kt, hf), load_half("k", kt, hf))))
                for s in range(NTC):
                    push_projqk(p, "k", s, kh)
                for hf in range(2):
                    for kt in range(KT8):
                        filler.append(
                            (80, None, lambda kt=kt, hf=hf:
                             qh.__setitem__((kt, hf), load_half("q", kt, hf))))
                for s in range(NTC):
                    push_projqk(p, "q", s, qh)

            queue_pair_prep(1)

            prev = None
            for pair in range(NP):
                for qt in range(NQT):
                    last = (pair == NP - 1 and qt == NQT - 1)
                    if last:
                        drain_all()
                    attn_qt(pair, qt, prev, own_pv_tail=last)
                    prev = (pair, qt)
                if pair == 0:
                    push_projv(1)   # V for head pairs 2-3, during pair 1
                if pair + 2 < NP:
                    queue_pair_prep(pair + 2)

            drain_all()
            pv_quarter(prev[0], prev[1], 0, 3)
            pv_quarter(prev[0], prev[1], 1, 3, pool=prps)

    nc.compile()
    return nc


_NC_CACHE = {}


def _get_nc():
    if "nc" not in _NC_CACHE:
        _NC_CACHE["nc"] = _build_nc()
    return _NC_CACHE["nc"]


def _pack_w(W):
    # [D, JW] -> [128, KT8*JW]: row p = concat over kt of W[kt*128+p, :]
    return np.ascontiguousarray(
        W.reshape(KT8, 128, JW).transpose(1, 0, 2).reshape(128, KT8 * JW)
    ).astype(_BF)


def _make_in_maps(key, value, query, Wq, bq, Wk, bk, Wv):
    in_maps = []
    for c in range(N_CORES):
        b, hh = c // 2, c % 2
        js = slice(hh * JW, (hh + 1) * JW)
        in_maps.append({
            "qT": np.ascontiguousarray(query[b].T).astype(_BF),
            "kT": np.ascontiguousarray(key[b].T).astype(_BF),
            "vT": np.ascontiguousarray(value[b].T).astype(_BF),
            "wq": _pack_w(Wq[:, js]),
            "wk": _pack_w(Wk[:, js]),
            "wv": _pack_w(Wv[:, js]),
            "bq": np.ascontiguousarray(bq[js], dtype=np.float32),
            "bk": np.ascontiguousarray(bk[js], dtype=np.float32),
        })
    return in_maps


def _assemble(results, bv):
    out = np.empty((B, S, H * DH), np.float32)
    for c in range(N_CORES):
        b, hh = c // 2, c % 2
        numT = results[c]["numT"]
        blk = numT[:, :DH, :] / numT[:, DH:DH + 1, :]
        out[b, :, hh * JW:(hh + 1) * JW] = (
            blk.reshape(JW, S).T + bv[hh * JW:(hh + 1) * JW])
    return out


def kernel(key, value, query, Wq, bq, Wk, bk, Wv, bv, **_run_kwargs):
    key = np.asarray(key, np.float32)
    value = np.asarray(value, np.float32)
    query = np.asarray(query, np.float32)
    nc = _get_nc()
    in_maps = _make_in_maps(key, value, query,
                            np.asarray(Wq, np.float32), np.asarray(bq, np.float32),
                            np.asarray(Wk, np.float32), np.asarray(bk, np.float32),
                            np.asarray(Wv, np.float32))
    res = run_bass_kernel_spmd(nc, in_maps, list(range(N_CORES)), **_run_kwargs)
    out = _assemble(res.results, np.asarray(bv, np.float32))
    if _run_kwargs:
        kernel.last_result = res
    return out


# revision 19
# speedup vs baseline: 1.0003x; 1.0003x over previous
"""Trainium2 Bass/Tile kernel for nn_MultiHeadAttention (B=4, S=2048, D=1024,
H=16, Dh=64, fp32), SPMD across 8 NeuronCores.

Sharding: core c -> batch c//2, head-half c%2 (8 heads per core).
Host pre-transposes each batch slice to [D, S], packs the weight slices to
[128, KT8*JW] (one 8KB-contiguous DMA row per partition instead of 1KB
bursts), and casts to bf16.  On device the QK projections produce Q^T/K^T
[feat, tok], the V projection produces V [tok, feat] with an appended
ones-column, scores come out as scores^T [k, q] (head pairs run
concurrently on the PE via 64-row tile positions), exp runs on the scalar
engine (scale 1/sqrt(Dh) folded in; scores bounded ~+-3 so no
max-subtraction), and the PV matmul uses V as stationary, yielding out^T
plus the softmax denominator from the ones column.  Host divides by the
denominator, adds the V bias, transposes, reassembles.

Schedule: the kernel is ACT(exp)-bound (~33.6M exps/core at 1 elem/lane/cyc
@1.2GHz plus ~430 cycles per instruction), so everything is organized
around keeping the scalar engine streaming:
  - scores ring: 2 PSUM tiles x 3 banks ([128, 3, 512] f32); each ACTIVATE
    covers 3 slabs (1536 elem/lane) to amortize the per-instruction cost.
  - score matmuls are emitted kt-pair at a time with ACTIVATEs deferred to
    kt boundaries, so the two 64-row head matmuls of each kt stay adjacent
    in the PE queue and execute concurrently (row tile positions 0/64).
  - PV chains are split into 4-matmul quarters emitted between score
    groups; projection work (incl. the next head-pair's, with its input
    re-DMA) is chopped into ~1us filler items consumed in leftover PE
    slots, with drain barriers before any emission that reads their output
    (emission order defines the dependency graph).
  - V-projection is computed in half-width chunks (head pairs 0-1 first,
    2-3 during pair 1) so the first PV chains aren't gated on the full V.
  - startup splits the inputs across both HW-DGE queues (k+v on sync, q on
    scalar, [128,1024] half tiles) and the exp table is preloaded.
PSUM: scores 6 banks, PV accumulator 1 bank, projection accumulator 1 bank
(shared with the final qt's second PV chain).
SBUF: Q^T/K^T live in a 2-slot ring (current pair + next being projected).
"""

import collections

import numpy as np
import ml_dtypes

import concourse.bacc as bacc
import concourse.tile as tile
from concourse import mybir
from concourse.bass_utils import run_bass_kernel_spmd

F32 = mybir.dt.float32
BF16 = mybir.dt.bfloat16
_BF = ml_dtypes.bfloat16

B, S, D, H, DH = 4, 2048, 1024, 16, 64
HH = 8          # heads per core
NP = HH // 2    # head pairs per core
JW = HH * DH    # 512 projected features per core
N_CORES = 8

KT8 = D // 128   # 8 contraction tiles for projections
NQT = S // 512   # 4 q-chunks of 512
NKT = S // 128   # 16 k-blocks of 128
NTT = S // 128   # 16 v token blocks
TC = 512         # projection token chunk
NTC = S // TC
HTOK = 1024      # k/q half-tile token width
GS = 3           # slabs per score PSUM tile / per ACTIVATE
NSLAB = 2 * NKT  # 32 score slabs per (pair, qt)
NG = (NSLAB + GS - 1) // GS   # 11 score tiles per (pair, qt)


def _build_nc(exp_bufs=18, in_bufs=28):
    nc = bacc.Bacc("TRN2", target_bir_lowering=False, debug=False,
                   num_devices=N_CORES)

    qT = nc.declare_dram_parameter("qT", [D, S], BF16, isOutput=False)
    kT = nc.declare_dram_parameter("kT", [D, S], BF16, isOutput=False)
    vT = nc.declare_dram_parameter("vT", [D, S], BF16, isOutput=False)
    # host-packed: [128, KT8*JW], row p = concat over kt of W[kt*128+p, :]
    wq = nc.declare_dram_parameter("wq", [128, KT8 * JW], BF16, isOutput=False)
    wk = nc.declare_dram_parameter("wk", [128, KT8 * JW], BF16, isOutput=False)
    wv = nc.declare_dram_parameter("wv", [128, KT8 * JW], BF16, isOutput=False)
    bq = nc.declare_dram_parameter("bq", [JW], F32, isOutput=False)
    bk = nc.declare_dram_parameter("bk", [JW], F32, isOutput=False)
    numT = nc.declare_dram_parameter("numT", [HH, 65, S], F32, isOutput=True)
    w_dram = {"wq": wq, "wk": wk, "wv": wv}
    in_dram = {"q": qT, "k": kT}

    with tile.TileContext(nc) as tc:
        with (
            tc.tile_pool(name="consts", bufs=1) as consts,
            tc.tile_pool(name="persist", bufs=1) as persist,
            tc.tile_pool(name="ins", bufs=in_bufs) as ins,
            tc.tile_pool(name="vins", bufs=8) as vins,
            tc.tile_pool(name="exps", bufs=exp_bufs) as exps,
            tc.tile_pool(name="ostage", bufs=4) as ostage,
            tc.tile_pool(name="scps", bufs=2, space="PSUM") as scps,
            tc.tile_pool(name="pvps", bufs=1, space="PSUM") as pvps,
            tc.tile_pool(name="prps", bufs=1, space="PSUM") as prps,
        ):
            w_sb = {}

            def load_w(name, eng=None):
                t = consts.tile([128, KT8, JW], BF16, tag=name)
                (eng or nc.sync).dma_start(
                    out=t[:].rearrange("p a b -> p (a b)"),
                    in_=w_dram[name].ap())
                w_sb[name] = t

            def load_bias(name, src):
                t = consts.tile([128, NP], F32, tag=name)
                nc.sync.dma_start(
                    out=t[:], in_=src.ap().rearrange("(pr j) -> j pr", j=128))
                return t

            # 2-slot ring: slot pair%2 holds that pair's projections
            QT_sb = persist.tile([128, 2, S], BF16, tag="QT")
            KT_sb = persist.tile([128, 2, S], BF16, tag="KT")
            V_aug = persist.tile([128, NTT, HH, 65], BF16, tag="Vaug")

            def load_half(name, kt, half, eng=None):
                t = ins.tile([128, HTOK], BF16, tag="in")
                (eng or nc.sync).dma_start(
                    out=t[:],
                    in_=in_dram[name].ap()[kt * 128:(kt + 1) * 128,
                                           half * HTOK:(half + 1) * HTOK])
                return t

            def load_vfull(kt):
                t = vins.tile([128, S], BF16, tag="vin")
                nc.sync.dma_start(
                    out=t[:], in_=vT.ap()[kt * 128:(kt + 1) * 128, :])
                return t

            # ---- ACT exp-table warm-up ----
            warm = persist.tile([128, 2], BF16, tag="warm")
            nc.vector.memset(warm[:, 0:1], 0.0)
            nc.scalar.activation(warm[:, 1:2], warm[:, 0:1],
                                 mybir.ActivationFunctionType.Exp, scale=0.125)

            # ---------------- projections ----------------
            proj_ps = {}

            def proj_qk_chunk(pair, name, s, halves, part):
                """part 0: MMs kt0-3; part 1: MMs kt4-7 + bias add."""
                wname, dst = {"k": ("wk", KT_sb), "q": ("wq", QT_sb)}[name]
                bias = bias_k if name == "k" else bias_q
                tc0 = s * TC
                hf, off = divmod(tc0, HTOK)
                if part == 0:
                    t = prps.tile([128, TC], F32, tag="pr",
                                  name=f"ps_{pair}_{name}_{s}")
                    proj_ps[(pair, name, s)] = t
                else:
                    t = proj_ps.pop((pair, name, s))
                for kt in range(part * 4, part * 4 + 4):
                    nc.tensor.matmul(
                        t[:], w_sb[wname][:, kt, pair * 128:(pair + 1) * 128],
                        halves[(kt, hf)][:, off:off + TC],
                        start=(kt == 0), stop=(kt == KT8 - 1))
                if part == 1:
                    nc.vector.tensor_scalar_add(
                        dst[:, pair % 2, tc0:tc0 + TC], t[:],
                        bias[:, pair:pair + 1])

            def proj_v_chunk(tt, ph, v_tiles):
                """V chunk for token block tt, head pairs 2ph..2ph+1."""
                t = prps.tile([128, 2 * 128], F32, tag="pr",
                              name=f"psv_{tt}_{ph}")
                for kt in range(KT8):
                    nc.tensor.matmul(
                        t[:], v_tiles[kt][:, tt * 128:(tt + 1) * 128],
                        w_sb["wv"][:, kt, ph * 256:(ph + 1) * 256],
                        start=(kt == 0), stop=(kt == KT8 - 1))
                nc.vector.tensor_copy(
                    V_aug[:, tt, ph * 4:(ph + 1) * 4, 0:64],
                    t[:].rearrange("p (h d) -> p h d", d=64))

            # ---------------- filler queue with drain barriers ----------
            filler = collections.deque()
            done_keys = set()

            def fill(budget_ns):
                while filler and budget_ns >= filler[0][0]:
                    cost, key, fn = filler.popleft()
                    fn()
                    if key:
                        done_keys.add(key)
                    budget_ns -= cost

            def drain(key):
                if key in done_keys:
                    return
                while filler:
                    cost, k, fn = filler.popleft()
                    fn()
                    if k:
                        done_keys.add(k)
                    if k == key:
                        return
                raise RuntimeError(f"drain: key {key} not found")

            def drain_all():
                while filler:
                    cost, k, fn = filler.popleft()
                    fn()
                    if k:
                        done_keys.add(k)

            # ---------------- attention ----------------
            et_map = {}     # (pair, qt, kt, h2) -> (et_tile, slot)
            pv_state = {}

            def pv_quarter(pair, qt, h2, quarter, pool=None):
                h = pair * 2 + h2
                q0 = qt * 512
                for kt in range(quarter * 4, quarter * 4 + 4):
                    drain(("vh", pair // 2, kt))
                if quarter == 0:
                    pv = (pool or pvps).tile(
                        [65, 512], F32, tag="pr" if pool else "pv",
                        name=f"pv_{pair}_{qt}_{h2}")
                    pv_state[(pair, qt, h2)] = pv
                else:
                    pv = pv_state[(pair, qt, h2)]
                for kt in range(quarter * 4, quarter * 4 + 4):
                    et, slot = et_map[(pair, qt, kt, h2)]
                    nc.tensor.matmul(
                        pv[:], V_aug[:, kt, h, :], et[:, slot, :],
                        start=(kt == 0), stop=(kt == NKT - 1))
                if quarter == 3:
                    del pv_state[(pair, qt, h2)]
                    ot = ostage.tile([65, 512], F32, tag="ot")
                    nc.vector.tensor_copy(ot[:], pv[:])
                    nc.sync.dma_start(
                        out=numT.ap()[h, :, q0:q0 + 512], in_=ot[:])
                    for kt in range(NKT):
                        del et_map[(pair, qt, kt, h2)]

            def emit_act(pair, qt, g, sc, n):
                et = exps.tile([128, GS, 512], BF16, tag="exp")
                nc.scalar.activation(
                    et[:, 0:n, :].rearrange("p a b -> p (a b)"),
                    sc[:].rearrange("p a b -> p (a b)"),
                    mybir.ActivationFunctionType.Exp, scale=0.125)
                for jj in range(n):
                    kk, hh2 = divmod(g * GS + jj, 2)
                    et_map[(pair, qt, kk, hh2)] = (et, jj)

            def attn_qt(pair, qt, prev, own_pv_tail=False):
                """Score matmuls kt-pair at a time (keeps the h0/h1 64-row
                matmuls adjacent -> concurrent); ACTIVATEs fire at kt
                boundaries; PV quarters of `prev` + filler fill the PE."""
                q0 = qt * 512
                drain(("q", pair, qt))
                sc_tiles = {}
                acted = 0
                slot = 0

                def do_slot():
                    nonlocal slot
                    if prev is not None and slot < 8:
                        pv_quarter(prev[0], prev[1], slot // 4, slot % 4)
                        fill(200)
                    elif own_pv_tail and slot >= 8:
                        qi = slot - 8
                        pv_quarter(pair, qt, 0, qi)
                        pv_quarter(pair, qt, 1, qi, pool=prps)
                    else:
                        fill(1250)
                    slot += 1

                for kt in range(NKT):
                    if kt % 4 == 0:
                        drain(("k", pair, kt // 4))
                    for h2 in range(2):
                        s = 2 * kt + h2
                        g, j = divmod(s, GS)
                        if j == 0:
                            n = min(GS, NSLAB - g * GS)
                            sc_tiles[g] = (scps.tile(
                                [128, n, 512], F32, tag="sc",
                                name=f"sc_{pair}_{qt}_{g}"), n)
                        nc.tensor.matmul(
                            sc_tiles[g][0][:, j, :],
                            KT_sb[h2 * 64:(h2 + 1) * 64, pair % 2,
                                  kt * 128:(kt + 1) * 128],
                            QT_sb[h2 * 64:(h2 + 1) * 64, pair % 2,
                                  q0:q0 + 512],
                            start=True, stop=True)
                    completed = (2 * kt + 2) // GS
                    while acted < completed:
                        sc, n = sc_tiles.pop(acted)
                        emit_act(pair, qt, acted, sc, n)
                        acted += 1
                        do_slot()
                while acted < NG:
                    sc, n = sc_tiles.pop(acted)
                    emit_act(pair, qt, acted, sc, n)
                    acted += 1
                    do_slot()

            # ---------------- emission ----------------
            # Keep both HW-DGE queues' startup bursts small: the DMA ring
            # credits are shared, so a long sync burst starves the scalar
            # queue's q stream (observed: q delayed ~15us behind k+v).
            # sync: wk + k half-0 only; scalar: wq + q half-0.  Everything
            # else (k half-1, wv, v, q half-1) issues as early filler items
            # on sync, in deadline order.
            load_w("wk")
            k_halves = {}
            for kt in range(KT8):
                k_halves[(kt, 0)] = load_half("k", kt, 0)
            bias_q = load_bias("bq", bq)
            bias_k = load_bias("bk", bk)
            load_w("wq", eng=nc.scalar)
            q_halves = {}
            for kt in range(KT8):
                q_halves[(kt, 0)] = load_half("q", kt, 0, eng=nc.scalar)
            v_tiles = {}

            nc.vector.memset(V_aug[:, :, :, 64:65], 1.0)

            # direct: K chunk s0 + Q chunk s0 -> first scores
            for part in range(2):
                proj_qk_chunk(0, "k", 0, k_halves, part)
            for part in range(2):
                proj_qk_chunk(0, "q", 0, q_halves, part)
            done_keys.add(("k", 0, 0))
            done_keys.add(("q", 0, 0))

            def push_projqk(pair, name, s, tiles):
                for part in range(2):
                    filler.append(
                        (950, (name, pair, s) if part else None,
                         lambda s=s, part=part, tiles=tiles, pair=pair,
                         name=name:
                         proj_qk_chunk(pair, name, s, tiles, part)))

            def push_projv(ph):
                for tt in range(NTT):
                    filler.append(
                        (700, ("vh", ph, tt),
                         lambda tt=tt, ph=ph: proj_v_chunk(tt, ph, v_tiles)))

            # pair-0 filler, deadline order: K s1 (k half-0), k half-1 DMAs,
            # K s2/s3, wv + v DMAs, Q s1, V half 0 chunks, q half-1 DMAs,
            # Q s2, Q s3
            push_projqk(0, "k", 1, k_halves)
            for kt in range(KT8):
                filler.append(
                    (80, None, lambda kt=kt:
                     k_halves.__setitem__((kt, 1), load_half("k", kt, 1))))
            push_projqk(0, "k", 2, k_halves)
            push_projqk(0, "k", 3, k_halves)
            filler.append((80, None, lambda: load_w("wv")))
            for kt in range(KT8):
                filler.append(
                    (80, None, lambda kt=kt:
                     v_tiles.__setitem__(kt, load_vfull(kt))))
            push_projqk(0, "q", 1, q_halves)
            push_projv(0)
            for kt in range(KT8):
                filler.append(
                    (80, None, lambda kt=kt:
                     q_halves.__setitem__((kt, 1), load_half("q", kt, 1))))
            push_projqk(0, "q", 2, q_halves)
            push_projqk(0, "q", 3, q_halves)

            def queue_pair_prep(p):
                """Input re-DMA + K/Q projections for pair p (filler).
                Urgent first (K s0/s1 + Q s0 -- what the pair-start drain
                pulls), lazy chunks after, so the drain burst at the pair
                boundary stays small; per-kt/per-qt drains pull the rest
                just in time."""
                kh, qh = {}, {}

                def dmas(name, store, hf):
                    for kt in range(KT8):
                        filler.append(
                            (80, None, lambda kt=kt:
                             store.__setitem__((kt, hf),
                                               load_half(name, kt, hf))))

                def urgent():
                    dmas("k", kh, 0)
                    push_projqk(p, "k", 0, kh)
                    push_projqk(p, "k", 1, kh)
                    dmas("q", qh, 0)
                    push_projqk(p, "q", 0, qh)

                def lazy():
                    dmas("k", kh, 1)
                    push_projqk(p, "k", 2, kh)
                    push_projqk(p, "k", 3, kh)
                    dmas("q", qh, 1)
                    push_projqk(p, "q", 1, qh)
                    push_projqk(p, "q", 2, qh)
                    push_projqk(p, "q", 3, qh)

                return urgent, lazy

            u1, l1 = queue_pair_prep(1)
            u1()
            l1()

            prev = None
            for pair in range(NP):
                for qt in range(NQT):
                    last = (pair == NP - 1 and qt == NQT - 1)
                    if last:
                        drain_all()
                    attn_qt(pair, qt, prev, own_pv_tail=last)
                    prev = (pair, qt)
                if pair == 0:
                    # for pair 2: urgent prep first (small pair-start drain),
                    # then V chunks for head pairs 2-3 (pulled per-quarter by
                    # the PV drains), then the lazy prep chunks
                    u2, l2 = queue_pair_prep(2)
                    u2()
                    push_projv(1)
                    l2()
                elif pair == 1:
                    u3, l3 = queue_pair_prep(3)
                    u3()
                    l3()

            drain_all()
            pv_quarter(prev[0], prev[1], 0, 3)
            pv_quarter(prev[0], prev[1], 1, 3, pool=prps)

    nc.compile()
    return nc


_NC_CACHE = {}


def _get_nc():
    if "nc" not in _NC_CACHE:
        _NC_CACHE["nc"] = _build_nc()
    return _NC_CACHE["nc"]


def _pack_w(W):
    # [D, JW] -> [128, KT8*JW]: row p = concat over kt of W[kt*128+p, :]
    return np.ascontiguousarray(
        W.reshape(KT8, 128, JW).transpose(1, 0, 2).reshape(128, KT8 * JW)
    ).astype(_BF)


def _make_in_maps(key, value, query, Wq, bq, Wk, bk, Wv):
    in_maps = []
    for c in range(N_CORES):
        b, hh = c // 2, c % 2
        js = slice(hh * JW, (hh + 1) * JW)
        in_maps.append({
            "qT": np.ascontiguousarray(query[b].T).astype(_BF),
            "kT": np.ascontiguousarray(key[b].T).astype(_BF),
            "vT": np.ascontiguousarray(value[b].T).astype(_BF),
            "wq": _pack_w(Wq[:, js]),
            "wk": _pack_w(Wk[:, js]),
            "wv": _pack_w(Wv[:, js]),
            "bq": np.ascontiguousarray(bq[js], dtype=np.float32),
            "bk": np.ascontiguousarray(bk[js], dtype=np.float32),
        })
    return in_maps


def _assemble(results, bv):
    out = np.empty((B, S, H * DH), np.float32)
    for c in range(N_CORES):
        b, hh = c // 2, c % 2
        numT = results[c]["numT"]
        blk = numT[:, :DH, :] / numT[:, DH:DH + 1, :]
        out[b, :, hh * JW:(hh + 1) * JW] = (
            blk.reshape(JW, S).T + bv[hh * JW:(hh + 1) * JW])
    return out


def kernel(key, value, query, Wq, bq, Wk, bk, Wv, bv, **_run_kwargs):
    key = np.asarray(key, np.float32)
    value = np.asarray(value, np.float32)
    query = np.asarray(query, np.float32)
    nc = _get_nc()
    in_maps = _make_in_maps(key, value, query,
                            np.asarray(Wq, np.float32), np.asarray(bq, np.float32),
                            np.asarray(Wk, np.float32), np.asarray(bk, np.float32),
                            np.asarray(Wv, np.float32))
    res = run_bass_kernel_spmd(nc, in_maps, list(range(N_CORES)), **_run_kwargs)
    out = _assemble(res.results, np.asarray(bv, np.float32))
    if _run_kwargs:
        kernel.last_result = res
    return out
